# revision 32
# baseline (speedup 1.0000x reference)
"""Multi-head causal self-attention (B=2, S=4096, D=512, H=8) on 8 trn2 cores.

Sharding: batch*heads = 16 (b,h) pairs -> 2 heads per core (head-parallel,
qkv weight columns sharded per head group). Zero cross-core communication.

Per-core kernel (heads h0=2g, h1=2g+1 stacked on partition halves):
  - inputs: xt = X[b].T  (512, 4096),  w = [Wq|Wk|Wv] head cols (512, 384)
  - QT/KT: (128, 4096) with partitions 0-63 = head0 dims, 64-127 = head1
  - V: natural layout per 128-row j-tile, with an appended ones column so
    the AV matmul also produces the softmax denominator.
  - scores computed transposed (keys on partitions) so softmax sum comes
    from the ones column; causal mask via 4 static mask tiles (diag only).
  - exp split across two engines: head0 (and all diagonal tiles) use the
    exact ACT exp; head1 off-diagonal tiles use a Schraudolph fast-exp on
    the DVE (i16 = round(A*s + B); bitcast i16 -> bf16 ~= exp(s/8)).
    Off-diagonal attention is diffuse, so the ~3% multiplicative error
    cancels through the softmax normalization (validated < 1e-4 effect).
  - AV accumulated in PSUM over j-tiles; result O.T (65, 512) transposed
    back via PE transpose in 128-col blocks; normalized with per-partition
    reciprocal of the denominator column; DMA'd out as full 512B rows.
  - PSUM->SBUF drains are split between ACT (Copy activations: qt/kt, O.T)
    and DVE (V-proj group copies, normalize) to keep both below PE's pace.
  - xt streams in per 512-column block; V projection for the j-tiles of
    range t+1 is prefetched inside range t, so compute starts after the
    first block instead of after the full 4MB load.
"""

import os
import sys

import numpy as np

for _p in ("/opt/trn_rl_repo", "/root/.axon_site/_ro/trn_rl_repo"):
    if os.path.isdir(_p) and _p not in sys.path:
        sys.path.append(_p)

import concourse.bass as bass
import concourse.tile as tile
from concourse import mybir
from concourse.masks import make_identity

F32 = mybir.dt.float32
BF16 = mybir.dt.bfloat16
FP16 = mybir.dt.float16
I16 = mybir.dt.int16

B, S, D, H = 2, 4096, 512, 8
HD = 64          # head dim
NHC = 2          # heads per core
P = 128          # partitions
KC = D // P      # 4 contraction chunks for the projection
IT = 512         # query-range width
NI = S // IT     # 8 query ranges
JT = 128         # key-tile width
NJ = S // JT     # 32 key tiles
SCALE = 1.0 / np.sqrt(HD)  # 0.125

# Schraudolph fast-exp constants (bf16 bit layout: 8exp/7mant).
# i16 = round(A*s_raw + B); bitcast(i16) ~= exp(s_raw/8) * (1 +- 3.2%)
A_SCH = (2.0 ** 7) / np.log(2.0) * SCALE
B_SCH = 16256.0 - 128.0 * np.log2(1.0614757) / 2.0 + 0.25

# EXP_MODE: 0 = all exp on ACT (no DVE offload)
#           1 = DVE tensor_scalar fp32->int16, AV rhs bitcast int16->bf16
#           2 = DVE tensor_scalar into a bf16 tile via write-side bitcast
EXP_MODE = int(os.environ.get("ATTN_EXP_MODE", "1"))


def build_nc():
    nc = bass.Bass()
    xt = nc.declare_dram_parameter("xt", [D, S], BF16, isOutput=False)
    w = nc.declare_dram_parameter("w", [D, 3 * P], BF16, isOutput=False)
    out = nc.declare_dram_parameter("out", [S, NHC * HD], F32, isOutput=True)

    exp_f = mybir.ActivationFunctionType.Exp
    copy_f = mybir.ActivationFunctionType.Copy
    mult_op = mybir.AluOpType.mult
    add_op = mybir.AluOpType.add

    with tile.TileContext(nc) as tc:
        with (
            tc.tile_pool(name="singles", bufs=1) as singles,
            tc.tile_pool(name="epool", bufs=6) as epool,
            tc.tile_pool(name="otpool", bufs=4) as otpool,
            tc.tile_pool(name="outtp", bufs=3) as outtp,
            tc.tile_pool(name="rcpool", bufs=4) as rcpool,
            tc.tile_pool(name="ps_sc", bufs=2, space="PSUM") as ps_sc,
            tc.tile_pool(name="ps_av", bufs=2, space="PSUM") as ps_av,
            tc.tile_pool(name="ps_tr", bufs=1, space="PSUM") as ps_tr,
            tc.tile_pool(name="ps_v", bufs=1, space="PSUM") as ps_v,
        ):
            # ---- resident tensors -------------------------------------
            xt_sb = singles.tile([P, KC, S], BF16, name="xt_sb")
            w_sb = singles.tile([P, KC, 3 * P], BF16, name="w_sb")
            qt = singles.tile([P, S], BF16, name="qt")
            kt = singles.tile([P, S], BF16, name="kt")
            # V per j-tile: [p, j, h, 0:64] values, [p, j, h, 64] ones
            v_sb = singles.tile([P, NJ, NHC, HD + 1], BF16, name="v_sb")
            masks_f = singles.tile([P, 4, IT], F32, name="masks_f")
            masks = singles.tile([P, 4, IT], BF16, name="masks")
            ident = singles.tile([P, P], F32, name="ident")
            ident_h = singles.tile([P, P], FP16, name="ident_h")
            zbias = singles.tile([P, 1], F32, name="zbias")

            # ---- loads + constants ------------------------------------
            nc.sync.dma_start(
                out=w_sb, in_=w[:, :].rearrange("(c p) n -> p c n", p=P)
            )
            xt_r = xt[:, :].rearrange("(c p) s -> p c s", p=P)
            for blk in range(NI):
                sl = slice(blk * IT, (blk + 1) * IT)
                for c in range(KC):
                    nc.sync.dma_start(out=xt_sb[:, c, sl], in_=xt_r[:, c, sl])

            make_identity(nc, ident)
            nc.vector.tensor_copy(ident_h, ident)
            nc.vector.memset(zbias, 0.0)
            # ones columns for the denominator rows
            nc.vector.memset(v_sb[:, :, :, HD : HD + 1], 1.0)
            # mask k: keep (=1) iff x - p - 128k >= 0, else 0
            for k in range(4):
                nc.gpsimd.memset(masks_f[:, k, :], 1.0)
                nc.gpsimd.affine_select(
                    out=masks_f[:, k, :],
                    in_=masks_f[:, k, :],
                    compare_op=mybir.AluOpType.is_ge,
                    fill=0.0,
                    base=-JT * k,
                    pattern=[[1, IT]],
                    channel_multiplier=-1,
                )
            nc.vector.tensor_copy(masks, masks_f)

            # ---- projections (emitted as per-pair "fill" units) -------
            # V proj for 2 j-tiles at a time: 8 small matmuls into the
            # shared 1-bank scratch PSUM, then a single strided DVE copy.
            def project_v_half(g, half):
                pv = ps_v.tile([P, 4 * P], F32, tag="vg", name="pv")
                j0 = 4 * g + 2 * half
                for jj in range(2):
                    for c in range(KC):
                        nc.tensor.matmul(
                            pv[:, jj * P : (jj + 1) * P],
                            lhsT=xt_sb[:, c, (j0 + jj) * JT : (j0 + jj + 1) * JT],
                            rhs=w_sb[:, c, 2 * P : 3 * P],
                            start=(c == 0),
                            stop=(c == KC - 1),
                        )
                nc.vector.tensor_copy(
                    v_sb[:, j0 : j0 + 2, :, 0:HD],
                    pv[:, 0 : 2 * P].rearrange(
                        "p (j h d) -> p j h d", j=2, h=NHC
                    ),
                )

            # QT or KT for one 512-query block: out = w.T @ xt_chunk
            def project_qk_half(r, which):
                sl = slice(r * IT, (r + 1) * IT)
                pq = ps_v.tile([P, 4 * P], F32, tag="vg", name="pq")
                for c in range(KC):
                    nc.tensor.matmul(
                        pq[:, 0:IT],
                        lhsT=w_sb[:, c, which * P : (which + 1) * P],
                        rhs=xt_sb[:, c, sl],
                        start=(c == 0),
                        stop=(c == KC - 1),
                    )
                dst = qt if which == 0 else kt
                nc.scalar.activation(dst[:, sl], pq[:, 0:IT], copy_f)

            # ---- attention --------------------------------------------
            # one tail unit = one (block, head): transpose + normalize; the
            # h==1 unit also DMAs the finished 128 output rows.  Units are
            # spread one-per-pair across the next range as PE/DVE/ACT fill.
            outt_live = {}

            def tail_unit(ot_tiles, ti0, blk, h, final=False):
                if h == 0:
                    outt_live[blk] = outtp.tile(
                        [P, NHC * HD], F32, tag="outt", name="out_t"
                    )
                out_t = outt_live[blk]
                trt = ps_tr.tile([P, HD + 1], FP16, tag="tr", name="tr",
                                 bufs=1)
                nc.tensor.transpose(
                    trt[:, 0 : HD + 1],
                    ot_tiles[h][:, blk * P : (blk + 1) * P],
                    ident_h[0 : HD + 1, 0 : HD + 1],
                )
                rc = rcpool.tile([P, 1], F32, tag="rc", name="rc")
                nc.vector.reciprocal(rc, trt[:, HD : HD + 1])
                if h == 0:
                    # normalize on ACT (Copy with per-partition scale) to
                    # keep DVE under its exp budget
                    nc.scalar.activation(
                        out_t[:, 0:HD], trt[:, 0:HD], copy_f, scale=rc
                    )
                else:
                    nc.vector.tensor_scalar_mul(
                        out_t[:, HD : 2 * HD], trt[:, 0:HD], rc
                    )
                    nc.sync.dma_start(
                        out=out[ti0 + blk * P : ti0 + (blk + 1) * P, :],
                        in_=out_t,
                    )
                    del outt_live[blk]

            # software-pipelined attention: scores for pair p+1 are emitted
            # before exp/AV of pair p, so the PE streams scores while the
            # ACT/DVE engines exponentiate the previous pair
            def pair_meta(p_i, njt):
                offs = []
                for u in (0, 1):
                    k = 2 * p_i + u - (njt - 4)
                    offs.append(JT * k if k > 0 else 0)
                return offs, 2 * p_i >= njt - 4

            def emit_scores(p_i, i0, njt):
                offs, diag = pair_meta(p_i, njt)
                sc = [
                    ps_sc.tile([P, 2 * IT], F32, tag="sc", name=f"sc{h}")
                    for h in range(NHC)
                ]
                e1_dt = BF16 if (diag or EXP_MODE != 1) else I16
                e = [
                    epool.tile([P, 2 * IT], BF16, tag="e", name="e0"),
                    epool.tile([P, 2 * IT], e1_dt, tag="e", name="e1"),
                ]
                for u in (0, 1):
                    j = 2 * p_i + u
                    for h in range(NHC):
                        hsl = slice(HD * h, HD * (h + 1))
                        nc.tensor.matmul(
                            sc[h][:, u * IT + offs[u] : (u + 1) * IT],
                            lhsT=kt[hsl, j * JT : (j + 1) * JT],
                            rhs=qt[hsl, i0 + offs[u] : i0 + IT],
                            start=True,
                            stop=True,
                            tile_position=(HD * h, 0),
                        )
                return sc, e, offs, diag

            def emit_exps(state, p_i, njt):
                sc, e, offs, diag = state
                if not diag:
                    # head0: exact exp on ACT; head1: Schraudolph on DVE
                    nc.scalar.activation(
                        e[0], sc[0], exp_f, bias=zbias, scale=SCALE
                    )
                    if EXP_MODE == 0:
                        nc.scalar.activation(
                            e[1], sc[1], exp_f, bias=zbias, scale=SCALE
                        )
                    else:
                        e1_out = e[1] if EXP_MODE == 1 else e[1].bitcast(I16)
                        nc.vector.tensor_scalar(
                            out=e1_out, in0=sc[1],
                            scalar1=float(A_SCH), scalar2=float(B_SCH),
                            op0=mult_op, op1=add_op,
                        )
                    return
                for h in range(NHC):
                    for u in (0, 1):
                        k = 2 * p_i + u - (njt - 4)
                        usl = slice(u * IT + offs[u], (u + 1) * IT)
                        nc.scalar.activation(
                            e[h][:, usl], sc[h][:, usl], exp_f,
                            bias=zbias, scale=SCALE,
                        )
                        if k >= 0:
                            nc.vector.tensor_mul(
                                e[h][:, usl], e[h][:, usl],
                                masks[:, k, offs[u] : IT],
                            )

            def emit_avs(state, av, p_i, njt):
                _, e, offs, diag = state
                for h in range(NHC):
                    for u in (0, 1):
                        j = 2 * p_i + u
                        rhs = e[h][:, u * IT + offs[u] : (u + 1) * IT]
                        if h == 1 and not diag and EXP_MODE == 1:
                            rhs = rhs.bitcast(BF16)
                        nc.tensor.matmul(
                            av[h][:, offs[u] : IT],
                            lhsT=v_sb[:, j, h, :],
                            rhs=rhs,
                            start=(j == 0),
                            stop=(j == njt - 1),
                        )

            pending = None            # (ot tiles, i0) of the previous range
            project_qk_half(0, 0)
            project_qk_half(0, 1)
            project_v_half(0, 0)
            project_v_half(0, 1)
            for t in range(NI):
                i0 = t * IT
                njt = 4 * (t + 1)         # causal: j-tiles 0..njt-1
                npairs = njt // 2
                av = [
                    ps_av.tile([HD + 1, IT], F32, tag="av", name=f"av{h}")
                    for h in range(NHC)
                ]
                # fill units for this range: next range's projections
                # (first — they gate range t+1) and the previous range's
                # output tails, spread roughly one per pair
                fills = []
                if t + 1 < NI:
                    fills.append(lambda r=t + 1: project_qk_half(r, 0))
                    fills.append(lambda r=t + 1: project_qk_half(r, 1))
                    fills.append(lambda r=t + 1: project_v_half(r, 0))
                    fills.append(lambda r=t + 1: project_v_half(r, 1))
                if pending is not None:
                    pt, pi0 = pending
                    for blk in range(IT // P):
                        for h in range(NHC):
                            fills.append(
                                lambda o=pt, i=pi0, b=blk, hh=h:
                                tail_unit(o, i, b, hh)
                            )
                    pending = None
                nf = len(fills)
                ndone = 0
                if t == 0:
                    state = emit_scores(0, i0, njt)
                for p_i in range(npairs):
                    emit_exps(state, p_i, njt)
                    # scores for the next pair BEFORE this pair's AV: both
                    # only wait on this pair's exps, and scores gate the
                    # NEXT exps — so emitting them first shortens the
                    # scores->exp recurrence by a full AV burst
                    if p_i + 1 < npairs:
                        next_state = emit_scores(p_i + 1, i0, njt)
                    elif t + 1 < NI:
                        next_state = emit_scores(0, i0 + IT, 4 * (t + 2))
                    else:
                        next_state = None
                    want = nf * (p_i + 1) // npairs
                    while ndone < want:
                        fills[ndone]()
                        ndone += 1
                    emit_avs(state, av, p_i, njt)
                    state = next_state
                # copy O.T out of PSUM (frees the av accumulators for the
                # next range); h0 on ACT, h1 on DVE so neither engine eats
                # both; transpose/normalize deferred into the next range
                ot = []
                for h in range(NHC):
                    # fp16: 10-bit mantissa keeps the O.T roundtrip error
                    # ~0.05%, and a 16-bit stationary operand makes the PE
                    # transpose stream at full rate (fp32 pays 4 cyc/row)
                    o = otpool.tile([HD + 1, IT], FP16, tag="ot",
                                    name=f"ot{h}")
                    if h == 0:
                        nc.scalar.activation(o, av[h], copy_f)
                    else:
                        nc.vector.tensor_copy(o, av[h])
                    ot.append(o)
                pending = (ot, i0)
            for blk in range(IT // P):
                for h in range(NHC):
                    tail_unit(pending[0], pending[1], blk, h, final=True)
    return nc


def legalize_waits(nc):
    """This toolchain's walrus allows at most ONE sync-wait per instruction;
    split extra waits onto preceding same-engine NoOps (same trick Tile uses
    for its own wait/update carriers)."""
    nsplit = 0
    for f in nc.m.functions:
        for blk in f.blocks:
            new_insts = []
            for inst in blk.instructions:
                si = getattr(inst, "sync_info", None)
                ow = list(si.on_wait) if (si is not None and si.on_wait) else []
                if len(ow) > 1:
                    for w_i, wcond in enumerate(ow[:-1]):
                        nsplit += 1
                        nop = mybir.InstNoOp(
                            name=f"{inst.name}-wsplit{w_i}",
                            sync_info=mybir.SyncInfo(on_wait=[wcond], on_update=[]),
                            bass_nofuse=True,
                            engine=inst.engine,
                        )
                        new_insts.append(nop)
                    si.on_wait = ow[-1:]
                new_insts.append(inst)
            try:
                blk.instructions[:] = new_insts
            except TypeError:
                blk.instructions = new_insts
    return nsplit


_NC_CACHE = None


def _get_nc():
    global _NC_CACHE
    if _NC_CACHE is None:
        nc = build_nc()
        legalize_waits(nc)
        _NC_CACHE = nc
    return _NC_CACHE


def shard_inputs(inputs, qkv_weights):
    import ml_dtypes

    bf16 = ml_dtypes.bfloat16
    x = np.ascontiguousarray(np.asarray(inputs, dtype=np.float32))
    wf = np.ascontiguousarray(np.asarray(qkv_weights, dtype=np.float32))
    in_maps = []
    for c in range(8):
        b, g = divmod(c, 4)
        lo = g * P
        xt_c = np.ascontiguousarray(x[b].T).astype(bf16)
        w_c = np.ascontiguousarray(
            np.concatenate(
                [wf[:, q * D + lo : q * D + lo + P] for q in range(3)], axis=1
            )
        ).astype(bf16)
        in_maps.append({"xt": xt_c, "w": w_c})
    return in_maps


def gather_outputs(results):
    out = np.empty((B, S, D), dtype=np.float32)
    for c in range(8):
        b, g = divmod(c, 4)
        out[b, :, g * P : (g + 1) * P] = results[c]["out"]
    return out


def run(in_maps, **kwargs):
    from concourse.bass_utils import run_bass_kernel_spmd

    return run_bass_kernel_spmd(_get_nc(), in_maps, list(range(8)), **kwargs)


def kernel(**inputs):
    in_maps = shard_inputs(inputs["inputs"], inputs["qkv_weights"])
    res = run(in_maps)
    return gather_outputs(res.results)


# revision 35
# speedup vs baseline: 1.0630x; 1.0630x over previous
"""Multi-head causal self-attention (B=2, S=4096, D=512, H=8) on 8 trn2 cores.

Sharding: batch*heads = 16 (b,h) pairs -> 2 heads per core (head-parallel,
qkv weight columns sharded per head group). Zero cross-core communication.

Per-core kernel (heads h0=2g, h1=2g+1 stacked on partition halves):
  - inputs: xt = X[b].T  (512, 4096),  w = [Wq|Wk|Wv] head cols (512, 384)
  - QT/KT: (128, 4096) with partitions 0-63 = head0 dims, 64-127 = head1
  - V: natural layout per 128-row j-tile, with an appended ones column so
    the AV matmul also produces the softmax denominator.
  - scores computed transposed (keys on partitions) so softmax sum comes
    from the ones column; causal mask via 4 static mask tiles (diag only).
  - exp split across two engines: head0 (and all diagonal tiles) use the
    exact ACT exp; head1 off-diagonal tiles use a Schraudolph fast-exp on
    the DVE (i16 = round(A*s + B); bitcast i16 -> bf16 ~= exp(s/8)).
    Off-diagonal attention is diffuse, so the ~3% multiplicative error
    cancels through the softmax normalization (validated < 1e-4 effect).
  - AV accumulated in PSUM over j-tiles; result O.T (65, 512) transposed
    back via PE transpose in 128-col blocks; normalized with per-partition
    reciprocal of the denominator column; DMA'd out as full 512B rows.
  - PSUM->SBUF drains are split between ACT (Copy activations: qt/kt, O.T)
    and DVE (V-proj group copies, normalize) to keep both below PE's pace.
  - xt streams in per 512-column block; V projection for the j-tiles of
    range t+1 is prefetched inside range t, so compute starts after the
    first block instead of after the full 4MB load.
"""

import os
import sys

import numpy as np

for _p in ("/opt/trn_rl_repo", "/root/.axon_site/_ro/trn_rl_repo"):
    if os.path.isdir(_p) and _p not in sys.path:
        sys.path.append(_p)

import concourse.bass as bass
import concourse.tile as tile
from concourse import mybir
from concourse.masks import make_identity

F32 = mybir.dt.float32
BF16 = mybir.dt.bfloat16
FP16 = mybir.dt.float16
I16 = mybir.dt.int16

B, S, D, H = 2, 4096, 512, 8
HD = 64          # head dim
NHC = 2          # heads per core
P = 128          # partitions
KC = D // P      # 4 contraction chunks for the projection
IT = 512         # query-range width
NI = S // IT     # 8 query ranges
JT = 128         # key-tile width
NJ = S // JT     # 32 key tiles
SCALE = 1.0 / np.sqrt(HD)  # 0.125

# Schraudolph fast-exp constants (bf16 bit layout: 8exp/7mant).
# i16 = round(A*s_raw + B); bitcast(i16) ~= exp(s_raw/8) * (1 +- 3.2%)
A_SCH = (2.0 ** 7) / np.log(2.0) * SCALE
B_SCH = 16256.0 - 128.0 * np.log2(1.0614757) / 2.0 + 0.25

# EXP_MODE: 0 = all exp on ACT (no DVE offload)
#           1 = DVE tensor_scalar fp32->int16, AV rhs bitcast int16->bf16
#           2 = DVE tensor_scalar into a bf16 tile via write-side bitcast
EXP_MODE = int(os.environ.get("ATTN_EXP_MODE", "1"))


def build_nc():
    nc = bass.Bass()
    xt = nc.declare_dram_parameter("xt", [D, S], BF16, isOutput=False)
    w = nc.declare_dram_parameter("w", [D, 3 * P], BF16, isOutput=False)
    out = nc.declare_dram_parameter("out", [S, NHC * HD], F32, isOutput=True)

    exp_f = mybir.ActivationFunctionType.Exp
    copy_f = mybir.ActivationFunctionType.Copy
    mult_op = mybir.AluOpType.mult
    add_op = mybir.AluOpType.add

    with tile.TileContext(nc) as tc:
        with (
            tc.tile_pool(name="singles", bufs=1) as singles,
            tc.tile_pool(name="epool", bufs=6) as epool,
            tc.tile_pool(name="otpool", bufs=4) as otpool,
            tc.tile_pool(name="outtp", bufs=3) as outtp,
            tc.tile_pool(name="rcpool", bufs=4) as rcpool,
            tc.tile_pool(name="ps_sc", bufs=2, space="PSUM") as ps_sc,
            tc.tile_pool(name="ps_av", bufs=2, space="PSUM") as ps_av,
            tc.tile_pool(name="ps_tr", bufs=1, space="PSUM") as ps_tr,
            tc.tile_pool(name="ps_v", bufs=1, space="PSUM") as ps_v,
        ):
            # ---- resident tensors -------------------------------------
            xt_sb = singles.tile([P, KC, S], BF16, name="xt_sb")
            w_sb = singles.tile([P, KC, 3 * P], BF16, name="w_sb")
            qt = singles.tile([P, S], BF16, name="qt")
            kt = singles.tile([P, S], BF16, name="kt")
            # partition-swapped replicas (head0 on partitions 64-127 and
            # vice versa): lets a lane's u0/u1 score matmuls use opposite
            # PE row groups and run concurrently
            qtb = singles.tile([P, S], BF16, name="qtb")
            ktb = singles.tile([P, S], BF16, name="ktb")
            # V per j-tile: [p, j, h, 0:64] values, [p, j, h, 64] ones
            v_sb = singles.tile([P, NJ, NHC, HD + 1], BF16, name="v_sb")
            masks_f = singles.tile([P, 4, IT], F32, name="masks_f")
            masks = singles.tile([P, 4, IT], BF16, name="masks")
            ident = singles.tile([P, P], F32, name="ident")
            ident_h = singles.tile([P, P], FP16, name="ident_h")
            zbias = singles.tile([P, 1], F32, name="zbias")

            # ---- loads + constants ------------------------------------
            nc.sync.dma_start(
                out=w_sb, in_=w[:, :].rearrange("(c p) n -> p c n", p=P)
            )
            xt_r = xt[:, :].rearrange("(c p) s -> p c s", p=P)
            for blk in range(NI):
                sl = slice(blk * IT, (blk + 1) * IT)
                for c in range(KC):
                    nc.sync.dma_start(out=xt_sb[:, c, sl], in_=xt_r[:, c, sl])

            make_identity(nc, ident)
            nc.vector.tensor_copy(ident_h, ident)
            nc.vector.memset(zbias, 0.0)
            # ones columns for the denominator rows
            nc.vector.memset(v_sb[:, :, :, HD : HD + 1], 1.0)
            # mask k: keep (=1) iff x - p - 128k >= 0, else 0
            for k in range(4):
                nc.gpsimd.memset(masks_f[:, k, :], 1.0)
                nc.gpsimd.affine_select(
                    out=masks_f[:, k, :],
                    in_=masks_f[:, k, :],
                    compare_op=mybir.AluOpType.is_ge,
                    fill=0.0,
                    base=-JT * k,
                    pattern=[[1, IT]],
                    channel_multiplier=-1,
                )
            nc.vector.tensor_copy(masks, masks_f)

            # ---- projections (emitted as per-pair "fill" units) -------
            # V proj for 2 j-tiles at a time: 8 small matmuls into the
            # shared 1-bank scratch PSUM, then a single strided DVE copy.
            def project_v_half(g, half):
                pv = ps_v.tile([P, 4 * P], F32, tag="vg", name="pv")
                j0 = 4 * g + 2 * half
                for jj in range(2):
                    for c in range(KC):
                        nc.tensor.matmul(
                            pv[:, jj * P : (jj + 1) * P],
                            lhsT=xt_sb[:, c, (j0 + jj) * JT : (j0 + jj + 1) * JT],
                            rhs=w_sb[:, c, 2 * P : 3 * P],
                            start=(c == 0),
                            stop=(c == KC - 1),
                        )
                nc.vector.tensor_copy(
                    v_sb[:, j0 : j0 + 2, :, 0:HD],
                    pv[:, 0 : 2 * P].rearrange(
                        "p (j h d) -> p j h d", j=2, h=NHC
                    ),
                )

            # QT or KT for one 512-query block: out = w.T @ xt_chunk
            def project_qk_half(r, which):
                sl = slice(r * IT, (r + 1) * IT)
                pq = ps_v.tile([P, 4 * P], F32, tag="vg", name="pq")
                for c in range(KC):
                    nc.tensor.matmul(
                        pq[:, 0:IT],
                        lhsT=w_sb[:, c, which * P : (which + 1) * P],
                        rhs=xt_sb[:, c, sl],
                        start=(c == 0),
                        stop=(c == KC - 1),
                    )
                dst, dstb = (qt, qtb) if which == 0 else (kt, ktb)
                nc.scalar.activation(dst[:, sl], pq[:, 0:IT], copy_f)
                # head-swapped replica via the idle DMA engine
                nc.sync.dma_start(out=dstb[HD:P, sl], in_=dst[0:HD, sl])
                nc.sync.dma_start(out=dstb[0:HD, sl], in_=dst[HD:P, sl])

            # ---- attention --------------------------------------------
            # one tail unit = one (block, head): transpose + normalize; the
            # h==1 unit also DMAs the finished 128 output rows.  Units are
            # spread one-per-pair across the next range as PE/DVE/ACT fill.
            outt_live = {}

            def tail_unit(ot_tiles, ti0, blk, h, final=False):
                if h == 0:
                    outt_live[blk] = outtp.tile(
                        [P, NHC * HD], F32, tag="outt", name="out_t"
                    )
                out_t = outt_live[blk]
                trt = ps_tr.tile([P, HD + 1], FP16, tag="tr", name="tr",
                                 bufs=1)
                nc.tensor.transpose(
                    trt[:, 0 : HD + 1],
                    ot_tiles[h][:, blk * P : (blk + 1) * P],
                    ident_h[0 : HD + 1, 0 : HD + 1],
                )
                rc = rcpool.tile([P, 1], F32, tag="rc", name="rc")
                nc.vector.reciprocal(rc, trt[:, HD : HD + 1])
                if h == 0:
                    # normalize on ACT (Copy with per-partition scale) to
                    # keep DVE under its exp budget
                    nc.scalar.activation(
                        out_t[:, 0:HD], trt[:, 0:HD], copy_f, scale=rc
                    )
                else:
                    nc.vector.tensor_scalar_mul(
                        out_t[:, HD : 2 * HD], trt[:, 0:HD], rc
                    )
                    nc.sync.dma_start(
                        out=out[ti0 + blk * P : ti0 + (blk + 1) * P, :],
                        in_=out_t,
                    )
                    del outt_live[blk]

            # software-pipelined attention: scores for pair p+1 are emitted
            # before exp/AV of pair p, so the PE streams scores while the
            # ACT/DVE engines exponentiate the previous pair
            def pair_meta(p_i, njt):
                offs = []
                for u in (0, 1):
                    k = 2 * p_i + u - (njt - 4)
                    offs.append(JT * k if k > 0 else 0)
                return offs, 2 * p_i >= njt - 4

            def emit_scores(p_i, i0, njt):
                offs, diag = pair_meta(p_i, njt)
                sc = [
                    ps_sc.tile([P, 2 * IT], F32, tag="sc", name=f"sc{h}")
                    for h in range(NHC)
                ]
                e1_dt = BF16 if (diag or EXP_MODE != 1) else I16
                e = [
                    epool.tile([P, 2 * IT], BF16, tag="e", name="e0"),
                    epool.tile([P, 2 * IT], e1_dt, tag="e", name="e1"),
                ]
                for h in range(NHC):
                    for u in (0, 1):
                        j = 2 * p_i + u
                        # u0 reads the natural-layout tiles, u1 the head-
                        # swapped replicas -> opposite row groups -> the
                        # lane's two score matmuls run concurrently
                        hh = h if u == 0 else 1 - h
                        kts, qts = (kt, qt) if u == 0 else (ktb, qtb)
                        hsl = slice(HD * hh, HD * (hh + 1))
                        nc.tensor.matmul(
                            sc[h][:, u * IT + offs[u] : (u + 1) * IT],
                            lhsT=kts[hsl, j * JT : (j + 1) * JT],
                            rhs=qts[hsl, i0 + offs[u] : i0 + IT],
                            start=True,
                            stop=True,
                            tile_position=(HD * hh, 0),
                        )
                return sc, e, offs, diag

            def emit_exps(state, p_i, njt):
                sc, e, offs, diag = state
                if not diag:
                    # head0: exact exp on ACT; head1: Schraudolph on DVE
                    nc.scalar.activation(
                        e[0], sc[0], exp_f, bias=zbias, scale=SCALE
                    )
                    if EXP_MODE == 0:
                        nc.scalar.activation(
                            e[1], sc[1], exp_f, bias=zbias, scale=SCALE
                        )
                    else:
                        e1_out = e[1] if EXP_MODE == 1 else e[1].bitcast(I16)
                        nc.vector.tensor_scalar(
                            out=e1_out, in0=sc[1],
                            scalar1=float(A_SCH), scalar2=float(B_SCH),
                            op0=mult_op, op1=add_op,
                        )
                    return
                for h in range(NHC):
                    for u in (0, 1):
                        k = 2 * p_i + u - (njt - 4)
                        usl = slice(u * IT + offs[u], (u + 1) * IT)
                        nc.scalar.activation(
                            e[h][:, usl], sc[h][:, usl], exp_f,
                            bias=zbias, scale=SCALE,
                        )
                        if k >= 0:
                            nc.vector.tensor_mul(
                                e[h][:, usl], e[h][:, usl],
                                masks[:, k, offs[u] : IT],
                            )

            def emit_avs(state, av, p_i, njt):
                _, e, offs, diag = state
                for h in range(NHC):
                    for u in (0, 1):
                        j = 2 * p_i + u
                        rhs = e[h][:, u * IT + offs[u] : (u + 1) * IT]
                        if h == 1 and not diag and EXP_MODE == 1:
                            rhs = rhs.bitcast(BF16)
                        nc.tensor.matmul(
                            av[h][:, offs[u] : IT],
                            lhsT=v_sb[:, j, h, :],
                            rhs=rhs,
                            start=(j == 0),
                            stop=(j == njt - 1),
                        )

            pending = None            # (ot tiles, i0) of the previous range
            project_qk_half(0, 0)
            project_qk_half(0, 1)
            project_v_half(0, 0)
            project_v_half(0, 1)
            for t in range(NI):
                i0 = t * IT
                njt = 4 * (t + 1)         # causal: j-tiles 0..njt-1
                npairs = njt // 2
                av = [
                    ps_av.tile([HD + 1, IT], F32, tag="av", name=f"av{h}")
                    for h in range(NHC)
                ]
                # fill units for this range: next range's projections
                # (first — they gate range t+1) and the previous range's
                # output tails, spread roughly one per pair
                fills = []
                if t + 1 < NI:
                    fills.append(lambda r=t + 1: project_qk_half(r, 0))
                    fills.append(lambda r=t + 1: project_qk_half(r, 1))
                    fills.append(lambda r=t + 1: project_v_half(r, 0))
                    fills.append(lambda r=t + 1: project_v_half(r, 1))
                if pending is not None:
                    pt, pi0 = pending
                    for blk in range(IT // P):
                        for h in range(NHC):
                            fills.append(
                                lambda o=pt, i=pi0, b=blk, hh=h:
                                tail_unit(o, i, b, hh)
                            )
                    pending = None
                nf = len(fills)
                ndone = 0
                if t == 0:
                    state = emit_scores(0, i0, njt)
                for p_i in range(npairs):
                    emit_exps(state, p_i, njt)
                    # scores for the next pair BEFORE this pair's AV: both
                    # only wait on this pair's exps, and scores gate the
                    # NEXT exps — so emitting them first shortens the
                    # scores->exp recurrence by a full AV burst
                    if p_i + 1 < npairs:
                        next_state = emit_scores(p_i + 1, i0, njt)
                    elif t + 1 < NI:
                        next_state = emit_scores(0, i0 + IT, 4 * (t + 2))
                    else:
                        next_state = None
                    want = nf * (p_i + 1) // npairs
                    while ndone < want:
                        fills[ndone]()
                        ndone += 1
                    emit_avs(state, av, p_i, njt)
                    state = next_state
                # copy O.T out of PSUM (frees the av accumulators for the
                # next range); h0 on ACT, h1 on DVE so neither engine eats
                # both; transpose/normalize deferred into the next range
                ot = []
                for h in range(NHC):
                    # fp16: 10-bit mantissa keeps the O.T roundtrip error
                    # ~0.05%, and a 16-bit stationary operand makes the PE
                    # transpose stream at full rate (fp32 pays 4 cyc/row)
                    o = otpool.tile([HD + 1, IT], FP16, tag="ot",
                                    name=f"ot{h}")
                    if h == 0:
                        nc.scalar.activation(o, av[h], copy_f)
                    else:
                        nc.vector.tensor_copy(o, av[h])
                    ot.append(o)
                pending = (ot, i0)
            for blk in range(IT // P):
                for h in range(NHC):
                    tail_unit(pending[0], pending[1], blk, h, final=True)
    return nc


def legalize_waits(nc):
    """This toolchain's walrus allows at most ONE sync-wait per instruction;
    split extra waits onto preceding same-engine NoOps (same trick Tile uses
    for its own wait/update carriers)."""
    nsplit = 0
    for f in nc.m.functions:
        for blk in f.blocks:
            new_insts = []
            for inst in blk.instructions:
                si = getattr(inst, "sync_info", None)
                ow = list(si.on_wait) if (si is not None and si.on_wait) else []
                if len(ow) > 1:
                    for w_i, wcond in enumerate(ow[:-1]):
                        nsplit += 1
                        nop = mybir.InstNoOp(
                            name=f"{inst.name}-wsplit{w_i}",
                            sync_info=mybir.SyncInfo(on_wait=[wcond], on_update=[]),
                            bass_nofuse=True,
                            engine=inst.engine,
                        )
                        new_insts.append(nop)
                    si.on_wait = ow[-1:]
                new_insts.append(inst)
            try:
                blk.instructions[:] = new_insts
            except TypeError:
                blk.instructions = new_insts
    return nsplit


_NC_CACHE = None


def _get_nc():
    global _NC_CACHE
    if _NC_CACHE is None:
        nc = build_nc()
        legalize_waits(nc)
        _NC_CACHE = nc
    return _NC_CACHE


def shard_inputs(inputs, qkv_weights):
    import ml_dtypes

    bf16 = ml_dtypes.bfloat16
    x = np.ascontiguousarray(np.asarray(inputs, dtype=np.float32))
    wf = np.ascontiguousarray(np.asarray(qkv_weights, dtype=np.float32))
    in_maps = []
    for c in range(8):
        b, g = divmod(c, 4)
        lo = g * P
        xt_c = np.ascontiguousarray(x[b].T).astype(bf16)
        w_c = np.ascontiguousarray(
            np.concatenate(
                [wf[:, q * D + lo : q * D + lo + P] for q in range(3)], axis=1
            )
        ).astype(bf16)
        in_maps.append({"xt": xt_c, "w": w_c})
    return in_maps


def gather_outputs(results):
    out = np.empty((B, S, D), dtype=np.float32)
    for c in range(8):
        b, g = divmod(c, 4)
        out[b, :, g * P : (g + 1) * P] = results[c]["out"]
    return out


def run(in_maps, **kwargs):
    from concourse.bass_utils import run_bass_kernel_spmd

    return run_bass_kernel_spmd(_get_nc(), in_maps, list(range(8)), **kwargs)


def kernel(**inputs):
    in_maps = shard_inputs(inputs["inputs"], inputs["qkv_weights"])
    res = run(in_maps)
    return gather_outputs(res.results)


# revision 38
# speedup vs baseline: 1.0757x; 1.0119x over previous
"""Multi-head causal self-attention (B=2, S=4096, D=512, H=8) on 8 trn2 cores.

Sharding: batch*heads = 16 (b,h) pairs -> 2 heads per core (head-parallel,
qkv weight columns sharded per head group). Zero cross-core communication.

Per-core kernel (heads h0=2g, h1=2g+1 stacked on partition halves):
  - inputs: xt = X[b].T  (512, 4096),  w = [Wq|Wk|Wv] head cols (512, 384)
  - QT/KT: (128, 4096) with partitions 0-63 = head0 dims, 64-127 = head1
  - V: natural layout per 128-row j-tile, with an appended ones column so
    the AV matmul also produces the softmax denominator.
  - scores computed transposed (keys on partitions) so softmax sum comes
    from the ones column; causal mask via 4 static mask tiles (diag only).
  - exp split across two engines: head0 (and all diagonal tiles) use the
    exact ACT exp; head1 off-diagonal tiles use a Schraudolph fast-exp on
    the DVE (i16 = round(A*s + B); bitcast i16 -> bf16 ~= exp(s/8)).
    Off-diagonal attention is diffuse, so the ~3% multiplicative error
    cancels through the softmax normalization (validated < 1e-4 effect).
  - AV accumulated in PSUM over j-tiles; result O.T (65, 512) transposed
    back via PE transpose in 128-col blocks; normalized with per-partition
    reciprocal of the denominator column; DMA'd out as full 512B rows.
  - PSUM->SBUF drains are split between ACT (Copy activations: qt/kt, O.T)
    and DVE (V-proj group copies, normalize) to keep both below PE's pace.
  - xt streams in per 512-column block; V projection for the j-tiles of
    range t+1 is prefetched inside range t, so compute starts after the
    first block instead of after the full 4MB load.
"""

import os
import sys

import numpy as np

for _p in ("/opt/trn_rl_repo", "/root/.axon_site/_ro/trn_rl_repo"):
    if os.path.isdir(_p) and _p not in sys.path:
        sys.path.append(_p)

import concourse.bass as bass
import concourse.tile as tile
from concourse import mybir
from concourse.masks import make_identity

F32 = mybir.dt.float32
BF16 = mybir.dt.bfloat16
FP16 = mybir.dt.float16
I16 = mybir.dt.int16

B, S, D, H = 2, 4096, 512, 8
HD = 64          # head dim
NHC = 2          # heads per core
P = 128          # partitions
KC = D // P      # 4 contraction chunks for the projection
IT = 512         # query-range width
NI = S // IT     # 8 query ranges
JT = 128         # key-tile width
NJ = S // JT     # 32 key tiles
SCALE = 1.0 / np.sqrt(HD)  # 0.125

# Schraudolph fast-exp constants (bf16 bit layout: 8exp/7mant).
# i16 = round(A*s_raw + B); bitcast(i16) ~= exp(s_raw/8) * (1 +- 3.2%)
A_SCH = (2.0 ** 7) / np.log(2.0) * SCALE
B_SCH = 16256.0 - 128.0 * np.log2(1.0614757) / 2.0 + 0.25

# EXP_MODE: 0 = all exp on ACT (no DVE offload)
#           1 = DVE tensor_scalar fp32->int16, AV rhs bitcast int16->bf16
#           2 = DVE tensor_scalar into a bf16 tile via write-side bitcast
EXP_MODE = int(os.environ.get("ATTN_EXP_MODE", "1"))


def build_nc():
    nc = bass.Bass()
    xt = nc.declare_dram_parameter("xt", [D, S], BF16, isOutput=False)
    w = nc.declare_dram_parameter("w", [D, 3 * P], BF16, isOutput=False)
    out = nc.declare_dram_parameter("out", [S, NHC * HD], F32, isOutput=True)

    exp_f = mybir.ActivationFunctionType.Exp
    copy_f = mybir.ActivationFunctionType.Copy
    mult_op = mybir.AluOpType.mult
    add_op = mybir.AluOpType.add

    with tile.TileContext(nc) as tc:
        with (
            tc.tile_pool(name="singles", bufs=1) as singles,
            tc.tile_pool(name="epool", bufs=6) as epool,
            tc.tile_pool(name="otpool", bufs=4) as otpool,
            tc.tile_pool(name="outtp", bufs=3) as outtp,
            tc.tile_pool(name="rcpool", bufs=4) as rcpool,
            tc.tile_pool(name="ps_sc", bufs=2, space="PSUM") as ps_sc,
            tc.tile_pool(name="ps_av", bufs=2, space="PSUM") as ps_av,
            tc.tile_pool(name="ps_tr", bufs=1, space="PSUM") as ps_tr,
            tc.tile_pool(name="ps_v", bufs=1, space="PSUM") as ps_v,
        ):
            # ---- resident tensors -------------------------------------
            xt_sb = singles.tile([P, KC, S], BF16, name="xt_sb")
            w_sb = singles.tile([P, KC, 3 * P], BF16, name="w_sb")
            qt = singles.tile([P, S], BF16, name="qt")
            kt = singles.tile([P, S], BF16, name="kt")
            # partition-swapped replicas (head0 on partitions 64-127 and
            # vice versa): lets a lane's u0/u1 score matmuls use opposite
            # PE row groups and run concurrently
            qtb = singles.tile([P, S], BF16, name="qtb")
            ktb = singles.tile([P, S], BF16, name="ktb")
            # V per j-tile: [p, j, h, 0:64] values, [p, j, h, 64] ones
            v_sb = singles.tile([P, NJ, NHC, HD + 1], BF16, name="v_sb")
            masks_f = singles.tile([P, 4, IT], F32, name="masks_f")
            masks = singles.tile([P, 4, IT], BF16, name="masks")
            ident = singles.tile([P, P], F32, name="ident")
            ident_h = singles.tile([P, P], FP16, name="ident_h")
            zbias = singles.tile([P, 1], F32, name="zbias")

            # ---- loads + constants ------------------------------------
            nc.sync.dma_start(
                out=w_sb, in_=w[:, :].rearrange("(c p) n -> p c n", p=P)
            )
            xt_r = xt[:, :].rearrange("(c p) s -> p c s", p=P)
            for blk in range(NI):
                sl = slice(blk * IT, (blk + 1) * IT)
                for c in range(KC):
                    # first blocks ride the second hwdge queue (ACT) so
                    # they land concurrently with the w load on sync
                    eng = nc.scalar if blk < 2 else nc.sync
                    eng.dma_start(out=xt_sb[:, c, sl], in_=xt_r[:, c, sl])

            make_identity(nc, ident)
            nc.vector.tensor_copy(ident_h, ident)
            nc.vector.memset(zbias, 0.0)
            # ones columns for the denominator rows
            nc.vector.memset(v_sb[:, :, :, HD : HD + 1], 1.0)
            # mask k: keep (=1) iff x - p - 128k >= 0, else 0
            for k in range(4):
                nc.gpsimd.memset(masks_f[:, k, :], 1.0)
                nc.gpsimd.affine_select(
                    out=masks_f[:, k, :],
                    in_=masks_f[:, k, :],
                    compare_op=mybir.AluOpType.is_ge,
                    fill=0.0,
                    base=-JT * k,
                    pattern=[[1, IT]],
                    channel_multiplier=-1,
                )
            nc.vector.tensor_copy(masks, masks_f)

            # ---- projections (emitted as per-pair "fill" units) -------
            # V proj for 2 j-tiles at a time: 8 small matmuls into the
            # shared 1-bank scratch PSUM, then a single strided DVE copy.
            def project_v_half(g, half):
                pv = ps_v.tile([P, 4 * P], F32, tag="vg", name="pv")
                j0 = 4 * g + 2 * half
                for jj in range(2):
                    for c in range(KC):
                        nc.tensor.matmul(
                            pv[:, jj * P : (jj + 1) * P],
                            lhsT=xt_sb[:, c, (j0 + jj) * JT : (j0 + jj + 1) * JT],
                            rhs=w_sb[:, c, 2 * P : 3 * P],
                            start=(c == 0),
                            stop=(c == KC - 1),
                        )
                nc.vector.tensor_copy(
                    v_sb[:, j0 : j0 + 2, :, 0:HD],
                    pv[:, 0 : 2 * P].rearrange(
                        "p (j h d) -> p j h d", j=2, h=NHC
                    ),
                )

            # QT or KT for one 512-query block: out = w.T @ xt_chunk
            def project_qk_half(r, which):
                sl = slice(r * IT, (r + 1) * IT)
                pq = ps_v.tile([P, 4 * P], F32, tag="vg", name="pq")
                for c in range(KC):
                    nc.tensor.matmul(
                        pq[:, 0:IT],
                        lhsT=w_sb[:, c, which * P : (which + 1) * P],
                        rhs=xt_sb[:, c, sl],
                        start=(c == 0),
                        stop=(c == KC - 1),
                    )
                dst, dstb = (qt, qtb) if which == 0 else (kt, ktb)
                nc.scalar.activation(dst[:, sl], pq[:, 0:IT], copy_f)
                # head-swapped replica via the idle DMA engine
                nc.sync.dma_start(out=dstb[HD:P, sl], in_=dst[0:HD, sl])
                nc.sync.dma_start(out=dstb[0:HD, sl], in_=dst[HD:P, sl])

            # ---- attention --------------------------------------------
            # one tail unit = one (block, head): transpose + normalize; the
            # h==1 unit also DMAs the finished 128 output rows.  Units are
            # spread one-per-pair across the next range as PE/DVE/ACT fill.
            outt_live = {}

            def tail_unit(ot_tiles, ti0, blk, h, final=False):
                if h == 0:
                    outt_live[blk] = outtp.tile(
                        [P, NHC * HD], F32, tag="outt", name="out_t"
                    )
                out_t = outt_live[blk]
                if final:
                    # scores are done — rotate through the two (2-bank)
                    # score slots so the last 8 transposes pipeline instead
                    # of serializing on the single ps_tr bank (fp32 ot
                    # tiles there, so dtypes line up with the f32 slots)
                    trt = ps_sc.tile([P, 2 * IT], F32, tag="sc", name="trf")
                    idn = ident
                else:
                    trt = ps_tr.tile([P, HD + 1], FP16, tag="tr", name="tr",
                                     bufs=1)
                    idn = ident_h
                nc.tensor.transpose(
                    trt[:, 0 : HD + 1],
                    ot_tiles[h][:, blk * P : (blk + 1) * P],
                    idn[0 : HD + 1, 0 : HD + 1],
                )
                rc = rcpool.tile([P, 1], F32, tag="rc", name="rc")
                nc.vector.reciprocal(rc, trt[:, HD : HD + 1])
                if h == 0:
                    # normalize on ACT (Copy with per-partition scale) to
                    # keep DVE under its exp budget
                    nc.scalar.activation(
                        out_t[:, 0:HD], trt[:, 0:HD], copy_f, scale=rc
                    )
                else:
                    nc.vector.tensor_scalar_mul(
                        out_t[:, HD : 2 * HD], trt[:, 0:HD], rc
                    )
                    nc.sync.dma_start(
                        out=out[ti0 + blk * P : ti0 + (blk + 1) * P, :],
                        in_=out_t,
                    )
                    del outt_live[blk]

            # software-pipelined attention: scores for pair p+1 are emitted
            # before exp/AV of pair p, so the PE streams scores while the
            # ACT/DVE engines exponentiate the previous pair
            def pair_meta(p_i, njt):
                offs = []
                for u in (0, 1):
                    k = 2 * p_i + u - (njt - 4)
                    offs.append(JT * k if k > 0 else 0)
                return offs, 2 * p_i >= njt - 4

            def emit_scores(p_i, i0, njt):
                offs, diag = pair_meta(p_i, njt)
                sc = [
                    ps_sc.tile([P, 2 * IT], F32, tag="sc", name=f"sc{h}")
                    for h in range(NHC)
                ]
                e1_dt = BF16 if (diag or EXP_MODE != 1) else I16
                e = [
                    epool.tile([P, 2 * IT], BF16, tag="e", name="e0"),
                    epool.tile([P, 2 * IT], e1_dt, tag="e", name="e1"),
                ]
                for h in range(NHC):
                    for u in (0, 1):
                        j = 2 * p_i + u
                        # u0 reads the natural-layout tiles, u1 the head-
                        # swapped replicas -> opposite row groups -> the
                        # lane's two score matmuls run concurrently
                        hh = h if u == 0 else 1 - h
                        kts, qts = (kt, qt) if u == 0 else (ktb, qtb)
                        hsl = slice(HD * hh, HD * (hh + 1))
                        nc.tensor.matmul(
                            sc[h][:, u * IT + offs[u] : (u + 1) * IT],
                            lhsT=kts[hsl, j * JT : (j + 1) * JT],
                            rhs=qts[hsl, i0 + offs[u] : i0 + IT],
                            start=True,
                            stop=True,
                            tile_position=(HD * hh, 0),
                        )
                return sc, e, offs, diag

            def emit_exps(state, p_i, njt):
                sc, e, offs, diag = state
                if not diag:
                    # head0: exact exp on ACT; head1: Schraudolph on DVE
                    nc.scalar.activation(
                        e[0], sc[0], exp_f, bias=zbias, scale=SCALE
                    )
                    if EXP_MODE == 0:
                        nc.scalar.activation(
                            e[1], sc[1], exp_f, bias=zbias, scale=SCALE
                        )
                    else:
                        e1_out = e[1] if EXP_MODE == 1 else e[1].bitcast(I16)
                        nc.vector.tensor_scalar(
                            out=e1_out, in0=sc[1],
                            scalar1=float(A_SCH), scalar2=float(B_SCH),
                            op0=mult_op, op1=add_op,
                        )
                    return
                for h in range(NHC):
                    for u in (0, 1):
                        k = 2 * p_i + u - (njt - 4)
                        usl = slice(u * IT + offs[u], (u + 1) * IT)
                        nc.scalar.activation(
                            e[h][:, usl], sc[h][:, usl], exp_f,
                            bias=zbias, scale=SCALE,
                        )
                        if k >= 0:
                            nc.vector.tensor_mul(
                                e[h][:, usl], e[h][:, usl],
                                masks[:, k, offs[u] : IT],
                            )

            def emit_avs(state, av, p_i, njt):
                _, e, offs, diag = state
                for h in range(NHC):
                    for u in (0, 1):
                        j = 2 * p_i + u
                        rhs = e[h][:, u * IT + offs[u] : (u + 1) * IT]
                        if h == 1 and not diag and EXP_MODE == 1:
                            rhs = rhs.bitcast(BF16)
                        nc.tensor.matmul(
                            av[h][:, offs[u] : IT],
                            lhsT=v_sb[:, j, h, :],
                            rhs=rhs,
                            start=(j == 0),
                            stop=(j == njt - 1),
                        )

            pending = None            # (ot tiles, i0) of the previous range
            project_qk_half(0, 0)
            project_qk_half(0, 1)
            project_v_half(0, 0)
            project_v_half(0, 1)
            for t in range(NI):
                i0 = t * IT
                njt = 4 * (t + 1)         # causal: j-tiles 0..njt-1
                npairs = njt // 2
                av = [
                    ps_av.tile([HD + 1, IT], F32, tag="av", name=f"av{h}")
                    for h in range(NHC)
                ]
                # fill units for this range: next range's projections
                # (first — they gate range t+1) and the previous range's
                # output tails, spread roughly one per pair
                fills = []
                if t + 1 < NI:
                    fills.append(lambda r=t + 1: project_qk_half(r, 0))
                    fills.append(lambda r=t + 1: project_qk_half(r, 1))
                    fills.append(lambda r=t + 1: project_v_half(r, 0))
                    fills.append(lambda r=t + 1: project_v_half(r, 1))
                if pending is not None:
                    pt, pi0 = pending
                    for blk in range(IT // P):
                        for h in range(NHC):
                            fills.append(
                                lambda o=pt, i=pi0, b=blk, hh=h:
                                tail_unit(o, i, b, hh)
                            )
                    pending = None
                nf = len(fills)
                ndone = 0
                if t == 0:
                    state = emit_scores(0, i0, njt)
                for p_i in range(npairs):
                    emit_exps(state, p_i, njt)
                    # scores for the next pair BEFORE this pair's AV: both
                    # only wait on this pair's exps, and scores gate the
                    # NEXT exps — so emitting them first shortens the
                    # scores->exp recurrence by a full AV burst
                    if p_i + 1 < npairs:
                        next_state = emit_scores(p_i + 1, i0, njt)
                    elif t + 1 < NI:
                        next_state = emit_scores(0, i0 + IT, 4 * (t + 2))
                    else:
                        next_state = None
                    want = nf * (p_i + 1) // npairs
                    while ndone < want:
                        fills[ndone]()
                        ndone += 1
                    emit_avs(state, av, p_i, njt)
                    state = next_state
                # copy O.T out of PSUM (frees the av accumulators for the
                # next range); h0 on ACT, h1 on DVE so neither engine eats
                # both; transpose/normalize deferred into the next range
                ot = []
                last = t == NI - 1
                for h in range(NHC):
                    # fp16: 10-bit mantissa keeps the O.T roundtrip error
                    # ~0.05%, and a 16-bit stationary operand makes the PE
                    # transpose stream at full rate (fp32 pays 4 cyc/row).
                    # The final range stays fp32 to match the score-slot
                    # PSUM its transposes rotate through.
                    o = otpool.tile(
                        [HD + 1, IT], F32 if last else FP16,
                        tag="otf" if last else "ot", bufs=2 if last else 4,
                        name=f"ot{h}",
                    )
                    if h == 0:
                        nc.scalar.activation(o, av[h], copy_f)
                    else:
                        nc.vector.tensor_copy(o, av[h])
                    ot.append(o)
                pending = (ot, i0)
            for blk in range(IT // P):
                for h in range(NHC):
                    tail_unit(pending[0], pending[1], blk, h, final=True)
    return nc


def legalize_waits(nc):
    """This toolchain's walrus allows at most ONE sync-wait per instruction;
    split extra waits onto preceding same-engine NoOps (same trick Tile uses
    for its own wait/update carriers)."""
    nsplit = 0
    for f in nc.m.functions:
        for blk in f.blocks:
            new_insts = []
            for inst in blk.instructions:
                si = getattr(inst, "sync_info", None)
                ow = list(si.on_wait) if (si is not None and si.on_wait) else []
                if len(ow) > 1:
                    for w_i, wcond in enumerate(ow[:-1]):
                        nsplit += 1
                        nop = mybir.InstNoOp(
                            name=f"{inst.name}-wsplit{w_i}",
                            sync_info=mybir.SyncInfo(on_wait=[wcond], on_update=[]),
                            bass_nofuse=True,
                            engine=inst.engine,
                        )
                        new_insts.append(nop)
                    si.on_wait = ow[-1:]
                new_insts.append(inst)
            try:
                blk.instructions[:] = new_insts
            except TypeError:
                blk.instructions = new_insts
    return nsplit


_NC_CACHE = None


def _get_nc():
    global _NC_CACHE
    if _NC_CACHE is None:
        nc = build_nc()
        legalize_waits(nc)
        _NC_CACHE = nc
    return _NC_CACHE


def shard_inputs(inputs, qkv_weights):
    import ml_dtypes

    bf16 = ml_dtypes.bfloat16
    x = np.ascontiguousarray(np.asarray(inputs, dtype=np.float32))
    wf = np.ascontiguousarray(np.asarray(qkv_weights, dtype=np.float32))
    in_maps = []
    for c in range(8):
        b, g = divmod(c, 4)
        lo = g * P
        xt_c = np.ascontiguousarray(x[b].T).astype(bf16)
        w_c = np.ascontiguousarray(
            np.concatenate(
                [wf[:, q * D + lo : q * D + lo + P] for q in range(3)], axis=1
            )
        ).astype(bf16)
        in_maps.append({"xt": xt_c, "w": w_c})
    return in_maps


def gather_outputs(results):
    out = np.empty((B, S, D), dtype=np.float32)
    for c in range(8):
        b, g = divmod(c, 4)
        out[b, :, g * P : (g + 1) * P] = results[c]["out"]
    return out


def run(in_maps, **kwargs):
    from concourse.bass_utils import run_bass_kernel_spmd

    return run_bass_kernel_spmd(_get_nc(), in_maps, list(range(8)), **kwargs)


def kernel(**inputs):
    in_maps = shard_inputs(inputs["inputs"], inputs["qkv_weights"])
    res = run(in_maps)
    return gather_outputs(res.results)


# revision 42
# speedup vs baseline: 1.0789x; 1.0030x over previous
"""Multi-head causal self-attention (B=2, S=4096, D=512, H=8) on 8 trn2 cores.

Sharding: batch*heads = 16 (b,h) pairs -> 2 heads per core (head-parallel,
qkv weight columns sharded per head group). Zero cross-core communication.

Per-core kernel (heads h0=2g, h1=2g+1 stacked on partition halves):
  - inputs: xt = X[b].T  (512, 4096),  w = [Wq|Wk|Wv] head cols (512, 384)
  - QT/KT: (128, 4096) with partitions 0-63 = head0 dims, 64-127 = head1
  - V: natural layout per 128-row j-tile, with an appended ones column so
    the AV matmul also produces the softmax denominator.
  - scores computed transposed (keys on partitions) so softmax sum comes
    from the ones column; causal mask via 4 static mask tiles (diag only).
  - exp split across two engines: head0 (and all diagonal tiles) use the
    exact ACT exp; head1 off-diagonal tiles use a Schraudolph fast-exp on
    the DVE (i16 = round(A*s + B); bitcast i16 -> bf16 ~= exp(s/8)).
    Off-diagonal attention is diffuse, so the ~3% multiplicative error
    cancels through the softmax normalization (validated < 1e-4 effect).
  - AV accumulated in PSUM over j-tiles; result O.T (65, 512) transposed
    back via PE transpose in 128-col blocks; normalized with per-partition
    reciprocal of the denominator column; DMA'd out as full 512B rows.
  - PSUM->SBUF drains are split between ACT (Copy activations: qt/kt, O.T)
    and DVE (V-proj group copies, normalize) to keep both below PE's pace.
  - xt streams in per 512-column block; V projection for the j-tiles of
    range t+1 is prefetched inside range t, so compute starts after the
    first block instead of after the full 4MB load.
"""

import os
import sys

import numpy as np

for _p in ("/opt/trn_rl_repo", "/root/.axon_site/_ro/trn_rl_repo"):
    if os.path.isdir(_p) and _p not in sys.path:
        sys.path.append(_p)

import concourse.bass as bass
import concourse.tile as tile
from concourse import mybir
from concourse.masks import make_identity

F32 = mybir.dt.float32
BF16 = mybir.dt.bfloat16
FP16 = mybir.dt.float16
I16 = mybir.dt.int16

B, S, D, H = 2, 4096, 512, 8
HD = 64          # head dim
NHC = 2          # heads per core
P = 128          # partitions
KC = D // P      # 4 contraction chunks for the projection
IT = 512         # query-range width
NI = S // IT     # 8 query ranges
JT = 128         # key-tile width
NJ = S // JT     # 32 key tiles
SCALE = 1.0 / np.sqrt(HD)  # 0.125

# Schraudolph fast-exp constants (bf16 bit layout: 8exp/7mant).
# i16 = round(A*s_raw + B); bitcast(i16) ~= exp(s_raw/8) * (1 +- 3.2%)
A_SCH = (2.0 ** 7) / np.log(2.0) * SCALE
B_SCH = 16256.0 - 128.0 * np.log2(1.0614757) / 2.0 + 0.25

# EXP_MODE: 0 = all exp on ACT (no DVE offload)
#           1 = DVE tensor_scalar fp32->int16, AV rhs bitcast int16->bf16
#           2 = DVE tensor_scalar into a bf16 tile via write-side bitcast
EXP_MODE = int(os.environ.get("ATTN_EXP_MODE", "1"))


def build_nc():
    nc = bass.Bass()
    xt = nc.declare_dram_parameter("xt", [D, S], BF16, isOutput=False)
    w = nc.declare_dram_parameter("w", [D, 3 * P], BF16, isOutput=False)
    out = nc.declare_dram_parameter("out", [S, NHC * HD], F32, isOutput=True)

    exp_f = mybir.ActivationFunctionType.Exp
    copy_f = mybir.ActivationFunctionType.Copy
    mult_op = mybir.AluOpType.mult
    add_op = mybir.AluOpType.add

    with tile.TileContext(nc) as tc:
        with (
            tc.tile_pool(name="singles", bufs=1) as singles,
            tc.tile_pool(name="epool", bufs=6) as epool,
            tc.tile_pool(name="otpool", bufs=4) as otpool,
            tc.tile_pool(name="outtp", bufs=3) as outtp,
            tc.tile_pool(name="rcpool", bufs=4) as rcpool,
            tc.tile_pool(name="ps_sc", bufs=2, space="PSUM") as ps_sc,
            tc.tile_pool(name="ps_av", bufs=2, space="PSUM") as ps_av,
            tc.tile_pool(name="ps_tr", bufs=1, space="PSUM") as ps_tr,
            tc.tile_pool(name="ps_v", bufs=1, space="PSUM") as ps_v,
        ):
            # ---- resident tensors -------------------------------------
            xt_sb = singles.tile([P, KC, S], BF16, name="xt_sb")
            w_sb = singles.tile([P, KC, 3 * P], BF16, name="w_sb")
            qt = singles.tile([P, S], BF16, name="qt")
            kt = singles.tile([P, S], BF16, name="kt")
            # partition-swapped replicas (head0 on partitions 64-127 and
            # vice versa): lets a lane's u0/u1 score matmuls use opposite
            # PE row groups and run concurrently
            qtb = singles.tile([P, S], BF16, name="qtb")
            ktb = singles.tile([P, S], BF16, name="ktb")
            # V per j-tile: [p, j, h, 0:64] values, [p, j, h, 64] ones
            v_sb = singles.tile([P, NJ, NHC, HD + 1], BF16, name="v_sb")
            masks_f = singles.tile([P, 4, IT], F32, name="masks_f")
            masks = singles.tile([P, 4, IT], BF16, name="masks")
            ident = singles.tile([P, P], F32, name="ident")
            ident_h = singles.tile([P, P], FP16, name="ident_h")
            zbias = singles.tile([P, 1], F32, name="zbias")

            # ---- loads + constants ------------------------------------
            nc.sync.dma_start(
                out=w_sb, in_=w[:, :].rearrange("(c p) n -> p c n", p=P)
            )
            xt_r = xt[:, :].rearrange("(c p) s -> p c s", p=P)
            for blk in range(NI):
                sl = slice(blk * IT, (blk + 1) * IT)
                # first blocks ride the second hwdge queue (ACT) so they
                # land concurrently with the w load on sync
                eng = nc.scalar if blk < 2 else nc.sync
                eng.dma_start(out=xt_sb[:, :, sl], in_=xt_r[:, :, sl])

            make_identity(nc, ident)
            nc.vector.tensor_copy(ident_h, ident)
            nc.vector.memset(zbias, 0.0)
            # ones columns for the denominator rows
            nc.vector.memset(v_sb[:, :, :, HD : HD + 1], 1.0)
            # mask k: keep (=1) iff x - p - 128k >= 0, else 0
            for k in range(4):
                nc.gpsimd.memset(masks_f[:, k, :], 1.0)
                nc.gpsimd.affine_select(
                    out=masks_f[:, k, :],
                    in_=masks_f[:, k, :],
                    compare_op=mybir.AluOpType.is_ge,
                    fill=0.0,
                    base=-JT * k,
                    pattern=[[1, IT]],
                    channel_multiplier=-1,
                )
            nc.vector.tensor_copy(masks, masks_f)

            # ---- projections (emitted as per-pair "fill" units) -------
            # V proj for 2 j-tiles at a time: 8 small matmuls into the
            # shared 1-bank scratch PSUM, then a single strided DVE copy.
            def project_v_half(g, half):
                pv = ps_v.tile([P, 4 * P], F32, tag="vg", name="pv")
                j0 = 4 * g + 2 * half
                for jj in range(2):
                    for c in range(KC):
                        nc.tensor.matmul(
                            pv[:, jj * P : (jj + 1) * P],
                            lhsT=xt_sb[:, c, (j0 + jj) * JT : (j0 + jj + 1) * JT],
                            rhs=w_sb[:, c, 2 * P : 3 * P],
                            start=(c == 0),
                            stop=(c == KC - 1),
                        )
                nc.vector.tensor_copy(
                    v_sb[:, j0 : j0 + 2, :, 0:HD],
                    pv[:, 0 : 2 * P].rearrange(
                        "p (j h d) -> p j h d", j=2, h=NHC
                    ),
                )

            # QT or KT for one 512-query block: out = w.T @ xt_chunk
            def project_qk_half(r, which):
                sl = slice(r * IT, (r + 1) * IT)
                pq = ps_v.tile([P, 4 * P], F32, tag="vg", name="pq")
                for c in range(KC):
                    nc.tensor.matmul(
                        pq[:, 0:IT],
                        lhsT=w_sb[:, c, which * P : (which + 1) * P],
                        rhs=xt_sb[:, c, sl],
                        start=(c == 0),
                        stop=(c == KC - 1),
                    )
                dst, dstb = (qt, qtb) if which == 0 else (kt, ktb)
                # copy on DVE: these fills land on the diagonal pairs,
                # where ACT is saturated by the 4 trimmed exps but DVE is
                # mostly idle
                nc.vector.tensor_copy(dst[:, sl], pq[:, 0:IT])
                # head-swapped replica via the idle DMA engine
                nc.sync.dma_start(out=dstb[HD:P, sl], in_=dst[0:HD, sl])
                nc.sync.dma_start(out=dstb[0:HD, sl], in_=dst[HD:P, sl])

            # ---- attention --------------------------------------------
            # one tail unit = one (block, head): transpose + normalize; the
            # h==1 unit also DMAs the finished 128 output rows.  Units are
            # spread one-per-pair across the next range as PE/DVE/ACT fill.
            outt_live = {}

            def tail_unit(ot_tiles, ti0, blk, h, final=False):
                if h == 0:
                    outt_live[blk] = outtp.tile(
                        [P, NHC * HD], F32, tag="outt", name="out_t"
                    )
                out_t = outt_live[blk]
                if final:
                    # scores are done — rotate through the two (2-bank)
                    # score slots so the last 8 transposes pipeline instead
                    # of serializing on the single ps_tr bank (fp32 ot
                    # tiles there, so dtypes line up with the f32 slots)
                    trt = ps_sc.tile([P, 2 * IT], F32, tag="sc", name="trf")
                    idn = ident
                else:
                    trt = ps_tr.tile([P, HD + 1], FP16, tag="tr", name="tr",
                                     bufs=1)
                    idn = ident_h
                nc.tensor.transpose(
                    trt[:, 0 : HD + 1],
                    ot_tiles[h][:, blk * P : (blk + 1) * P],
                    idn[0 : HD + 1, 0 : HD + 1],
                )
                rc = rcpool.tile([P, 1], F32, tag="rc", name="rc")
                nc.vector.reciprocal(rc, trt[:, HD : HD + 1])
                if h == 0:
                    # normalize on ACT (Copy with per-partition scale) to
                    # keep DVE under its exp budget
                    nc.scalar.activation(
                        out_t[:, 0:HD], trt[:, 0:HD], copy_f, scale=rc
                    )
                else:
                    nc.vector.tensor_scalar_mul(
                        out_t[:, HD : 2 * HD], trt[:, 0:HD], rc
                    )
                    # final stores ride the (empty-by-then) second queue so
                    # the kernel isn't waiting on a backed-up sync queue
                    deng = nc.scalar if final else nc.sync
                    deng.dma_start(
                        out=out[ti0 + blk * P : ti0 + (blk + 1) * P, :],
                        in_=out_t,
                    )
                    del outt_live[blk]

            # software-pipelined attention: scores for pair p+1 are emitted
            # before exp/AV of pair p, so the PE streams scores while the
            # ACT/DVE engines exponentiate the previous pair
            def pair_meta(p_i, njt):
                offs = []
                for u in (0, 1):
                    k = 2 * p_i + u - (njt - 4)
                    offs.append(JT * k if k > 0 else 0)
                return offs, 2 * p_i >= njt - 4

            def emit_scores(p_i, i0, njt):
                offs, diag = pair_meta(p_i, njt)
                sc = [
                    ps_sc.tile([P, 2 * IT], F32, tag="sc", name=f"sc{h}")
                    for h in range(NHC)
                ]
                e1_dt = BF16 if (diag or EXP_MODE != 1) else I16
                e = [
                    epool.tile([P, 2 * IT], BF16, tag="e", name="e0"),
                    epool.tile([P, 2 * IT], e1_dt, tag="e", name="e1"),
                ]
                for h in range(NHC):
                    for u in (0, 1):
                        j = 2 * p_i + u
                        # u0 reads the natural-layout tiles, u1 the head-
                        # swapped replicas -> opposite row groups -> the
                        # lane's two score matmuls run concurrently
                        hh = h if u == 0 else 1 - h
                        kts, qts = (kt, qt) if u == 0 else (ktb, qtb)
                        hsl = slice(HD * hh, HD * (hh + 1))
                        nc.tensor.matmul(
                            sc[h][:, u * IT + offs[u] : (u + 1) * IT],
                            lhsT=kts[hsl, j * JT : (j + 1) * JT],
                            rhs=qts[hsl, i0 + offs[u] : i0 + IT],
                            start=True,
                            stop=True,
                            tile_position=(HD * hh, 0),
                        )
                return sc, e, offs, diag

            def emit_exps(state, p_i, njt):
                sc, e, offs, diag = state
                if not diag:
                    # head0: exact exp on ACT; head1: Schraudolph on DVE
                    nc.scalar.activation(
                        e[0], sc[0], exp_f, bias=zbias, scale=SCALE
                    )
                    if EXP_MODE == 0:
                        nc.scalar.activation(
                            e[1], sc[1], exp_f, bias=zbias, scale=SCALE
                        )
                    else:
                        e1_out = e[1] if EXP_MODE == 1 else e[1].bitcast(I16)
                        nc.vector.tensor_scalar(
                            out=e1_out, in0=sc[1],
                            scalar1=float(A_SCH), scalar2=float(B_SCH),
                            op0=mult_op, op1=add_op,
                        )
                    return
                for h in range(NHC):
                    for u in (0, 1):
                        k = 2 * p_i + u - (njt - 4)
                        usl = slice(u * IT + offs[u], (u + 1) * IT)
                        nc.scalar.activation(
                            e[h][:, usl], sc[h][:, usl], exp_f,
                            bias=zbias, scale=SCALE,
                        )
                        if k >= 0:
                            nc.vector.tensor_mul(
                                e[h][:, usl], e[h][:, usl],
                                masks[:, k, offs[u] : IT],
                            )

            def emit_avs(state, av, p_i, njt):
                _, e, offs, diag = state
                for h in range(NHC):
                    for u in (0, 1):
                        j = 2 * p_i + u
                        rhs = e[h][:, u * IT + offs[u] : (u + 1) * IT]
                        if h == 1 and not diag and EXP_MODE == 1:
                            rhs = rhs.bitcast(BF16)
                        nc.tensor.matmul(
                            av[h][:, offs[u] : IT],
                            lhsT=v_sb[:, j, h, :],
                            rhs=rhs,
                            start=(j == 0),
                            stop=(j == njt - 1),
                        )

            pending = None            # (ot tiles, i0) of the previous range
            project_qk_half(0, 0)
            project_qk_half(0, 1)
            project_v_half(0, 0)
            project_v_half(0, 1)
            for t in range(NI):
                i0 = t * IT
                njt = 4 * (t + 1)         # causal: j-tiles 0..njt-1
                npairs = njt // 2
                av = [
                    ps_av.tile([HD + 1, IT], F32, tag="av", name=f"av{h}")
                    for h in range(NHC)
                ]
                # fill units for this range: the previous range's output
                # tails first, then the next range's projections — the
                # projections land on the trailing (diagonal) pairs where
                # ACT is exp-saturated but the PE and DVE have slack
                fills = []
                if pending is not None:
                    pt, pi0 = pending
                    for blk in range(IT // P):
                        for h in range(NHC):
                            fills.append(
                                lambda o=pt, i=pi0, b=blk, hh=h:
                                tail_unit(o, i, b, hh)
                            )
                    pending = None
                if t + 1 < NI:
                    fills.append(lambda r=t + 1: project_qk_half(r, 0))
                    fills.append(lambda r=t + 1: project_qk_half(r, 1))
                    fills.append(lambda r=t + 1: project_v_half(r, 0))
                    fills.append(lambda r=t + 1: project_v_half(r, 1))
                nf = len(fills)
                ndone = 0
                if t == 0:
                    state = emit_scores(0, i0, njt)
                for p_i in range(npairs):
                    emit_exps(state, p_i, njt)
                    # scores for the next pair BEFORE this pair's AV: both
                    # only wait on this pair's exps, and scores gate the
                    # NEXT exps — so emitting them first shortens the
                    # scores->exp recurrence by a full AV burst
                    if p_i + 1 < npairs:
                        next_state = emit_scores(p_i + 1, i0, njt)
                    elif t + 1 < NI:
                        next_state = emit_scores(0, i0 + IT, 4 * (t + 2))
                    else:
                        next_state = None
                    want = nf * (p_i + 1) // npairs
                    while ndone < want:
                        fills[ndone]()
                        ndone += 1
                    emit_avs(state, av, p_i, njt)
                    state = next_state
                # copy O.T out of PSUM (frees the av accumulators for the
                # next range); h0 on ACT, h1 on DVE so neither engine eats
                # both; transpose/normalize deferred into the next range
                ot = []
                last = t == NI - 1
                for h in range(NHC):
                    # fp16: 10-bit mantissa keeps the O.T roundtrip error
                    # ~0.05%, and a 16-bit stationary operand makes the PE
                    # transpose stream at full rate (fp32 pays 4 cyc/row).
                    # The final range stays fp32 to match the score-slot
                    # PSUM its transposes rotate through.
                    o = otpool.tile(
                        [HD + 1, IT], F32 if last else FP16,
                        tag="otf" if last else "ot", bufs=2 if last else 4,
                        name=f"ot{h}",
                    )
                    if h == 0:
                        nc.scalar.activation(o, av[h], copy_f)
                    else:
                        nc.vector.tensor_copy(o, av[h])
                    ot.append(o)
                pending = (ot, i0)
            for blk in range(IT // P):
                for h in range(NHC):
                    tail_unit(pending[0], pending[1], blk, h, final=True)
    return nc


def legalize_waits(nc):
    """This toolchain's walrus allows at most ONE sync-wait per instruction;
    split extra waits onto preceding same-engine NoOps (same trick Tile uses
    for its own wait/update carriers)."""
    nsplit = 0
    for f in nc.m.functions:
        for blk in f.blocks:
            new_insts = []
            for inst in blk.instructions:
                si = getattr(inst, "sync_info", None)
                ow = list(si.on_wait) if (si is not None and si.on_wait) else []
                if len(ow) > 1:
                    for w_i, wcond in enumerate(ow[:-1]):
                        nsplit += 1
                        nop = mybir.InstNoOp(
                            name=f"{inst.name}-wsplit{w_i}",
                            sync_info=mybir.SyncInfo(on_wait=[wcond], on_update=[]),
                            bass_nofuse=True,
                            engine=inst.engine,
                        )
                        new_insts.append(nop)
                    si.on_wait = ow[-1:]
                new_insts.append(inst)
            try:
                blk.instructions[:] = new_insts
            except TypeError:
                blk.instructions = new_insts
    return nsplit


_NC_CACHE = None


def _get_nc():
    global _NC_CACHE
    if _NC_CACHE is None:
        nc = build_nc()
        legalize_waits(nc)
        _NC_CACHE = nc
    return _NC_CACHE


def shard_inputs(inputs, qkv_weights):
    import ml_dtypes

    bf16 = ml_dtypes.bfloat16
    x = np.ascontiguousarray(np.asarray(inputs, dtype=np.float32))
    wf = np.ascontiguousarray(np.asarray(qkv_weights, dtype=np.float32))
    in_maps = []
    for c in range(8):
        b, g = divmod(c, 4)
        lo = g * P
        xt_c = np.ascontiguousarray(x[b].T).astype(bf16)
        w_c = np.ascontiguousarray(
            np.concatenate(
                [wf[:, q * D + lo : q * D + lo + P] for q in range(3)], axis=1
            )
        ).astype(bf16)
        in_maps.append({"xt": xt_c, "w": w_c})
    return in_maps


def gather_outputs(results):
    out = np.empty((B, S, D), dtype=np.float32)
    for c in range(8):
        b, g = divmod(c, 4)
        out[b, :, g * P : (g + 1) * P] = results[c]["out"]
    return out


def run(in_maps, **kwargs):
    from concourse.bass_utils import run_bass_kernel_spmd

    return run_bass_kernel_spmd(_get_nc(), in_maps, list(range(8)), **kwargs)


def kernel(**inputs):
    in_maps = shard_inputs(inputs["inputs"], inputs["qkv_weights"])
    res = run(in_maps)
    return gather_outputs(res.results)
